# revision 9
# baseline (speedup 1.0000x reference)
"""Causal self-attention Trainium2 kernel (fused, bf16 + fp8 delta attention).

Problem: B=4, T=2048, D=2048, H=16 heads x 128 head-size, fp32.
Sharding: 8 cores = 4 batches x 2 head-groups (8 heads each).

Per core, fully fused in SBUF (no DRAM spills):
  A: qT/kT = (x@w + b)^T in bf16, v in bf16(x32) + fp8(x32) + fp8 residual
  B: causal attention per head:
     S = kT^T q (bf16) -> exm64 = exp(S*scale + ln64) bf16
     delta8 = exm64 - 64 (fp8, exactly -64 at masked positions)
     OT psum (scale 2048*ex*v) = sum_offdiag DoubleRow(delta8 @ v8)
                               + sum_diag   exm64 @ vbf32
     FC psum (scale 32*sum v) = DoubleRow ones matmuls over v8 + r8 (residual)
     den psum = DoubleRow ones @ delta8 (exact -64 cancellation at masks)
     ot = (OT/64 + FC) * (1/(32*den))  -> o_sb bf16
  C: out = o_sb @ wo (bf16), f32 out.
Host sums the two group partials per batch and adds (b_v@w_o + b_o).
"""

import sys

sys.path.insert(0, "/opt/trn_rl_repo")

import numpy as np
import ml_dtypes

import concourse.bass as bass
import concourse.bacc as bacc
import concourse.mybir as mybir
from concourse.tile import TileContext
from concourse.bass_utils import run_bass_kernel_spmd

DT = mybir.dt
AF = mybir.ActivationFunctionType
ALU = mybir.AluOpType
PM = mybir.MatmulPerfMode

B, T, D = 4, 2048, 2048
HPC = 8                 # heads per core
DH = 128                # head size
KT = D // 128           # 16 contraction tiles
TQ = T // 512           # 4 query chunks of 512
TT = T // 128           # 16 t tiles
SCALE = 1.0 / np.sqrt(DH)
LN64 = float(np.log(64.0))
NEG = -1e10


def build_nc(reps=1):
    nc = bacc.Bacc("TRN2", target_bir_lowering=False, debug=False)
    f32 = DT.float32
    f32r = DT.float32r
    bf16 = DT.bfloat16
    fp8 = DT.float8e4

    xt = nc.dram_tensor("xt", [KT, 128, T], bf16, kind="ExternalInput")
    wq = nc.dram_tensor("wq", [HPC, 128, KT, 128], bf16, kind="ExternalInput")
    wk = nc.dram_tensor("wk", [HPC, 128, KT, 128], bf16, kind="ExternalInput")
    wv = nc.dram_tensor("wv", [HPC // 2, 128, KT, 256], bf16, kind="ExternalInput")
    wo = nc.dram_tensor("wo", [128, HPC, D], bf16, kind="ExternalInput")
    bq = nc.dram_tensor("bq", [128, HPC], f32, kind="ExternalInput")
    bk = nc.dram_tensor("bk", [128, HPC], f32, kind="ExternalInput")
    maskadd = nc.dram_tensor("maskadd", [128, 1024], f32, kind="ExternalInput")
    ones8 = nc.dram_tensor("ones8", [128, 2, 128], fp8, kind="ExternalInput")
    ones64 = nc.dram_tensor("ones64", [128, 2, 2], fp8, kind="ExternalInput")
    onesr = nc.dram_tensor("onesr", [1, 128], f32r, kind="ExternalInput")
    outp = nc.dram_tensor("out", [T, D], f32, kind="ExternalOutput")

    def emit_rep(tc):
        import contextlib
        es = contextlib.ExitStack()
        with es:
            xt_pool = es.enter_context(tc.tile_pool(name="xt_pool", bufs=1))
            const_pool = es.enter_context(tc.tile_pool(name="const_pool", bufs=1))
            wqk_pool = es.enter_context(tc.tile_pool(name="wqk_pool", bufs=2))
            wv_pool = es.enter_context(tc.tile_pool(name="wv_pool", bufs=1))
            qk_pool = es.enter_context(tc.tile_pool(name="qk_pool", bufs=1))
            v8_pool = es.enter_context(tc.tile_pool(name="v8_pool", bufs=1))
            r8_pool = es.enter_context(tc.tile_pool(name="r8_pool", bufs=1))
            vbf_pool = es.enter_context(tc.tile_pool(name="vbf_pool", bufs=1))
            exm_pool = es.enter_context(tc.tile_pool(name="exm_pool", bufs=4))
            d8_pool = es.enter_context(tc.tile_pool(name="d8_pool", bufs=4))
            stg_pool = es.enter_context(tc.tile_pool(name="stg_pool", bufs=2))
            sm_pool = es.enter_context(tc.tile_pool(name="sm_pool", bufs=2))
            o_pool = es.enter_context(tc.tile_pool(name="o_pool", bufs=1))
            # ---- constants ------------------------------------------------
            bq_sb = const_pool.tile([128, HPC], f32)
            bk_sb = const_pool.tile([128, HPC], f32)
            mask_sb = const_pool.tile([128, 1024], f32)
            ones8_sb = const_pool.tile([128, 2, 128], fp8)
            ones64_sb = const_pool.tile([128, 2, 2], fp8)
            onesr_sb = const_pool.tile([1, 128], f32r)
            ln64_sb = const_pool.tile([128, 1], f32)
            nc.vector.memset(ln64_sb[:], LN64)
            onesb_sb = const_pool.tile([128, 128], bf16)
            nc.vector.memset(onesb_sb[:], 1.0)
            nc.sync.dma_start(out=bq_sb[:], in_=bq.ap())
            nc.sync.dma_start(out=bk_sb[:], in_=bk.ap())
            nc.sync.dma_start(out=mask_sb[:], in_=maskadd.ap())
            nc.sync.dma_start(out=ones8_sb[:], in_=ones8.ap())
            nc.sync.dma_start(out=ones64_sb[:], in_=ones64.ap())
            nc.sync.dma_start(out=onesr_sb[:], in_=onesr.ap())

            xts = []
            for a in range(KT):
                xta = xt_pool.tile([128, T], bf16, tag=f"xt{a}", name=f"xt{a}")
                nc.sync.dma_start(out=xta[:], in_=xt.ap()[a])
                xts.append(xta)

            o_sb = o_pool.tile([128, HPC, T], bf16)

            es2 = contextlib.ExitStack()
            with es2:
                ps_a = es2.enter_context(tc.tile_pool(name="ps_a", bufs=2, space="PSUM"))
                ps_s = es2.enter_context(tc.tile_pool(name="ps_s", bufs=2, space="PSUM"))
                ps_ot = es2.enter_context(tc.tile_pool(name="ps_ot", bufs=2, space="PSUM"))
                ps_db = es2.enter_context(tc.tile_pool(name="ps_db", bufs=1, space="PSUM"))
                ps_fc = es2.enter_context(tc.tile_pool(name="ps_fc", bufs=1, space="PSUM"))
                for pr in range(HPC // 2):
                    heads = (2 * pr, 2 * pr + 1)
                    qk_tiles = {}
                    # ---- A: q/k projections for both heads ----------------
                    for h in heads:
                        for wnm, w_dram, b_sb in (
                            ("q", wq, bq_sb), ("k", wk, bk_sb)
                        ):
                            wcol = wqk_pool.tile(
                                [128, KT, 128], bf16, tag="wcol"
                            )
                            nc.sync.dma_start(out=wcol[:], in_=w_dram.ap()[h])
                            dst = qk_pool.tile(
                                [128, T], bf16, tag=f"{wnm}T{h % 2}"
                            )
                            for c in range(TQ):
                                ps = ps_a.tile([128, 512], f32, tag="psa")
                                for a in range(KT):
                                    nc.tensor.matmul(
                                        ps[:],
                                        wcol[:, a, :],
                                        xts[a][:, c * 512:(c + 1) * 512],
                                        start=(a == 0),
                                        stop=(a == KT - 1),
                                    )
                                nc.scalar.activation(
                                    dst[:, c * 512:(c + 1) * 512], ps[:],
                                    AF.Identity, bias=b_sb[:, h:h + 1],
                                )
                            qk_tiles[(wnm, h)] = dst

                    # ---- A: v projection for the pair ---------------------
                    wvq = wv_pool.tile([128, KT, 256], bf16, tag="wvq")
                    nc.sync.dma_start(out=wvq[:], in_=wv.ap()[pr])
                    v8 = v8_pool.tile([128, TT, 256], fp8, tag="v8")
                    r8 = r8_pool.tile([128, TT, 256], fp8, tag="r8")
                    vbf = vbf_pool.tile([128, TT, 256], bf16, tag="vbf")
                    for tt in range(TT):
                        psf = ps_a.tile([128, 512], f32, tag="psa")
                        ps = psf[:, 0:256]
                        for a in range(KT):
                            nc.tensor.matmul(
                                ps,
                                xts[a][:, tt * 128:(tt + 1) * 128],
                                wvq[:, a, :],
                                start=(a == 0),
                                stop=(a == KT - 1),
                            )
                        with nc.allow_low_precision(
                            reason="fp8 v with explicit residual correction"
                        ):
                            nc.scalar.activation(
                                v8[:, tt, :], ps, AF.Identity, scale=32.0
                            )
                            nc.scalar.activation(
                                vbf[:, tt, :], ps, AF.Identity, scale=32.0
                            )
                            stg = stg_pool.tile([128, 256], bf16, tag="rstg")
                            # stg = v8/32 - v ;  r8 = -2048*stg = 2048*(v - v8/32)
                            nc.vector.scalar_tensor_tensor(
                                stg[:], v8[:, tt, :], 1.0 / 32.0, ps,
                                op0=ALU.mult, op1=ALU.subtract,
                            )
                            nc.vector.tensor_scalar_mul(
                                r8[:, tt, :], stg[:], -2048.0
                            )

                    # ---- B: attention per head ----------------------------
                    for h in heads:
                        qT = qk_tiles[("q", h)]
                        kT = qk_tiles[("k", h)]
                        hs = slice((h % 2) * 128, (h % 2) * 128 + 128)
                        for c in range(TQ):
                            ntk = 4 * (c + 1)
                            nd = 4 * c  # num strictly-below-diagonal tiles
                            qt = qT[:, c * 512:(c + 1) * 512]
                            otp = ps_ot.tile([128, 512], f32, tag="otp")
                            dbt = ps_db.tile([128, 512], f32, tag="dbt")
                            d8t = None
                            for j in range(ntk):
                                sp = ps_s.tile([128, 512], f32, tag="sp")
                                nc.tensor.matmul(
                                    sp[:],
                                    kT[:, j * 128:(j + 1) * 128],
                                    qt,
                                    start=True,
                                    stop=True,
                                )
                                d = j * 128 - c * 512
                                if d >= 0:
                                    nc.vector.tensor_add(
                                        sp[:], sp[:],
                                        mask_sb[:, 512 - d:1024 - d],
                                    )
                                exm = exm_pool.tile([128, 512], bf16, tag="exm")
                                nc.scalar.activation(
                                    exm[:], sp[:], AF.Exp,
                                    scale=SCALE, bias=ln64_sb[:],
                                )
                                if j < nd:
                                    if j % 2 == 0:
                                        d8t = d8_pool.tile(
                                            [128, 2, 512], fp8, tag="d8"
                                        )
                                    with nc.allow_low_precision(
                                        reason="fp8 softmax deltas by design"
                                    ):
                                        nc.vector.tensor_scalar_sub(
                                            d8t[:, j % 2, :], exm[:], 64.0
                                        )
                                if j >= nd:
                                    # diagonal tile: direct bf16 exm64 @ vbf32
                                    nc.tensor.matmul(
                                        otp[:],
                                        vbf[:, j, hs],
                                        exm[:],
                                        start=(j == 0),
                                        stop=(j == ntk - 1),
                                    )
                                if j % 2 == 1 and j < nd:
                                    # off-diag pair: fp8 DoubleRow
                                    nc.tensor.matmul(
                                        otp[:],
                                        v8[:, j - 1:j + 1, hs],
                                        d8t[:],
                                        start=(j == 1),
                                        stop=False,
                                        perf_mode=PM.DoubleRow,
                                    )
                                    nc.tensor.matmul(
                                        dbt[:, :],
                                        ones8_sb[:],
                                        d8t[:],
                                        start=(j == 1),
                                        stop=False,
                                        perf_mode=PM.DoubleRow,
                                    )
                                if j >= nd:
                                    # diag den: 64*sum(ex) via bf16 ones
                                    nc.tensor.matmul(
                                        dbt[:, :],
                                        onesb_sb[:],
                                        exm[:],
                                        start=(j == 0),
                                        stop=(j == ntk - 1),
                                    )
                            # FC: 32*sum(v) over kpos < 512c via v8 + r8
                            fc_sb = None
                            if nd > 0:
                                fcp = ps_fc.tile([128, 2], f32, tag="fcp")
                                for jj in range(0, nd, 2):
                                    nc.tensor.matmul(
                                        fcp[:],
                                        v8[:, jj:jj + 2, hs],
                                        ones8_sb[:, :, 0:2],
                                        start=(jj == 0),
                                        stop=False,
                                        perf_mode=PM.DoubleRow,
                                    )
                                for jj in range(0, nd, 2):
                                    nc.tensor.matmul(
                                        fcp[:],
                                        r8[:, jj:jj + 2, hs],
                                        ones64_sb[:],
                                        start=False,
                                        stop=(jj == nd - 2),
                                        perf_mode=PM.DoubleRow,
                                    )
                                fc_sb = sm_pool.tile([128, 1], f32, tag="fc")
                                nc.vector.tensor_copy(fc_sb[:], fcp[:, 0:1])
                            # den_sb = 32*den = 32*512*(c+1) + sum(d8)/2
                            den_sb = sm_pool.tile([1, 512], f32, tag="den")
                            nc.vector.tensor_scalar(
                                den_sb[:], dbt[0:1, :], 0.5,
                                float(32.0 * 512.0 * c),
                                op0=ALU.mult, op1=ALU.add,
                            )
                            rec = sm_pool.tile([1, 512], f32r, tag="rec")
                            with nc.allow_low_precision(
                                reason="f32r softmax reciprocal as in baseline"
                            ):
                                nc.vector.reciprocal(rec[:], den_sb[:])
                            # bc broadcast of rec into full dbt psum tile
                            nc.tensor.matmul(
                                dbt[:], onesr_sb[:], rec[:],
                                start=True, stop=True,
                            )
                            # t1 = otp/64 + fc  (= 32 * unnormalized ot)
                            t1 = stg_pool.tile([128, 512], f32, tag="t1")
                            if fc_sb is not None:
                                nc.vector.tensor_scalar(
                                    t1[:], otp[:], 1.0 / 64.0, fc_sb[:],
                                    op0=ALU.mult, op1=ALU.add,
                                )
                            else:
                                nc.vector.tensor_scalar_mul(
                                    t1[:], otp[:], 1.0 / 64.0
                                )
                            with nc.allow_low_precision(
                                reason="bf16 attention output by design"
                            ):
                                nc.vector.tensor_mul(
                                    o_sb[:, h, c * 512:(c + 1) * 512],
                                    t1[:], dbt[:],
                                )

            # ---- C: out = o @ wo ------------------------------------------
            es3 = contextlib.ExitStack()
            with es3:
                wo_pool = es3.enter_context(tc.tile_pool(name="wo_pool", bufs=2))
                co_stage = es3.enter_context(tc.tile_pool(name="co_stage", bufs=3))
                ps_c = es3.enter_context(tc.tile_pool(name="ps_c", bufs=2, space="PSUM"))
                for dc in range(4):
                    wot = wo_pool.tile([128, HPC, 512], bf16, tag="wot")
                    nc.sync.dma_start(
                        out=wot[:], in_=wo.ap()[:, :, dc * 512:(dc + 1) * 512]
                    )
                    for tt in range(TT):
                        psc = ps_c.tile([128, 512], f32, tag="psc")
                        for h in range(HPC):
                            nc.tensor.matmul(
                                psc[:],
                                o_sb[:, h, tt * 128:(tt + 1) * 128],
                                wot[:, h, :],
                                start=(h == 0),
                                stop=(h == HPC - 1),
                            )
                        stg = co_stage.tile([128, 512], f32, tag="cstg")
                        nc.scalar.activation(stg[:], psc[:], AF.Copy)
                        nc.sync.dma_start(
                            out=outp.ap()[tt * 128:(tt + 1) * 128,
                                          dc * 512:(dc + 1) * 512],
                            in_=stg[:],
                        )

    with TileContext(nc) as tc:
        for _rep in range(reps):
            emit_rep(tc)

    nc.compile()
    return nc


_NC_CACHE = {}


def _get_nc():
    if "nc" not in _NC_CACHE:
        _NC_CACHE["nc"] = build_nc()
    return _NC_CACHE["nc"]


def make_in_maps(query, w_q, b_q, w_k, b_k, w_v, b_v, w_o, b_o):
    query = np.asarray(query, dtype=np.float32)
    w_q = np.asarray(w_q, dtype=np.float32)
    w_k = np.asarray(w_k, dtype=np.float32)
    w_v = np.asarray(w_v, dtype=np.float32)
    w_o = np.asarray(w_o, dtype=np.float32)
    b_q = np.asarray(b_q, dtype=np.float32)
    b_k = np.asarray(b_k, dtype=np.float32)

    bf = ml_dtypes.bfloat16
    f8 = ml_dtypes.float8_e4m3

    g_idx = np.arange(1024)[None, :] - 512
    p_idx = np.arange(128)[:, None]
    maskadd = np.where(g_idx >= p_idx, 0.0, NEG).astype(np.float32)
    ones8 = np.ones((128, 2, 128), dtype=f8)
    ones64 = np.full((128, 2, 2), 1.0 / 64.0, dtype=f8)
    onesr = np.ones((1, 128), dtype=np.float32)

    in_maps = []
    for core in range(8):
        b = core // 2
        g = core % 2
        s = slice(g * 1024, (g + 1) * 1024)
        # xt[a, p, t] = query[b, t, 128a+p]
        xt = np.ascontiguousarray(
            query[b].T.reshape(KT, 128, T).astype(bf)
        )
        # wq[h, p, a, c] = w_q[128a+p, g*1024+128h+c]
        def prep_qk(w):
            wg = w[:, s].reshape(KT, 128, HPC, 128)  # [a, p, h, c]
            return np.ascontiguousarray(
                wg.transpose(2, 1, 0, 3).astype(bf)
            )
        # wv[pr, p, a, c] = w_v[128a+p, g*1024+256pr+c]
        wvg = w_v[:, s].reshape(KT, 128, HPC // 2, 256)
        wv_p = np.ascontiguousarray(wvg.transpose(2, 1, 0, 3).astype(bf))
        # wo[p, h, d] = w_o[g*1024+128h+p, d]
        wog = w_o[s, :].reshape(HPC, 128, D)
        wo_p = np.ascontiguousarray(wog.transpose(1, 0, 2).astype(bf))
        in_maps.append(
            {
                "xt": xt,
                "wq": prep_qk(w_q),
                "wk": prep_qk(w_k),
                "wv": wv_p,
                "wo": wo_p,
                "bq": np.ascontiguousarray(b_q[s].reshape(HPC, 128).T),
                "bk": np.ascontiguousarray(b_k[s].reshape(HPC, 128).T),
                "maskadd": maskadd,
                "ones8": ones8,
                "ones64": ones64,
                "onesr": onesr,
            }
        )

    return in_maps


def kernel(query, w_q, b_q, w_k, b_k, w_v, b_v, w_o, b_o, **kwargs):
    w_o = np.asarray(w_o, dtype=np.float32)
    b_v = np.asarray(b_v, dtype=np.float32)
    b_o = np.asarray(b_o, dtype=np.float32)
    in_maps = make_in_maps(query, w_q, b_q, w_k, b_k, w_v, b_v, w_o, b_o)
    global _LAST_IN_MAPS
    _LAST_IN_MAPS = in_maps
    nc = _get_nc()
    res = run_bass_kernel_spmd(nc, in_maps, core_ids=list(range(8)))

    out = np.zeros((B, T, D), dtype=np.float32)
    for core in range(8):
        out[core // 2] += res.results[core]["out"]
    out += (b_v @ w_o + b_o)[None, None, :]
    return out


# revision 10
# speedup vs baseline: 1.0564x; 1.0564x over previous
"""Causal self-attention Trainium2 kernel (fused, bf16 + fp8 delta attention).

Problem: B=4, T=2048, D=2048, H=16 heads x 128 head-size, fp32.
Sharding: 8 cores = 4 batches x 2 head-groups (8 heads each).

Per core, fully fused in SBUF (no DRAM spills):
  A: qT/kT = (x@w + b)^T in bf16, v in bf16(x32) + fp8(x32) + fp8 residual
  B: causal attention per head:
     S = kT^T q (bf16) -> exm64 = exp(S*scale + ln64) bf16
     delta8 = exm64 - 64 (fp8, exactly -64 at masked positions)
     OT psum (scale 2048*ex*v) = sum_offdiag DoubleRow(delta8 @ v8)
                               + sum_diag   exm64 @ vbf32
     FC psum (scale 32*sum v) = DoubleRow ones matmuls over v8 + r8 (residual)
     den psum = DoubleRow ones @ delta8 (exact -64 cancellation at masks)
     ot = (OT/64 + FC) * (1/(32*den))  -> o_sb bf16
  C: out = o_sb @ wo (bf16), f32 out.
Host sums the two group partials per batch and adds (b_v@w_o + b_o).
"""

import sys

sys.path.insert(0, "/opt/trn_rl_repo")

import numpy as np
import ml_dtypes

import concourse.bass as bass
import concourse.bacc as bacc
import concourse.mybir as mybir
from concourse.tile import TileContext
from concourse.bass_utils import run_bass_kernel_spmd

DT = mybir.dt
AF = mybir.ActivationFunctionType
ALU = mybir.AluOpType
PM = mybir.MatmulPerfMode

B, T, D = 4, 2048, 2048
HPC = 8                 # heads per core
DH = 128                # head size
KT = D // 128           # 16 contraction tiles
TQ = T // 512           # 4 query chunks of 512
TT = T // 128           # 16 t tiles
SCALE = 1.0 / np.sqrt(DH)
LN64 = float(np.log(64.0))
NEG = -1e10


def build_nc(reps=1):
    nc = bacc.Bacc("TRN2", target_bir_lowering=False, debug=False)
    f32 = DT.float32
    f32r = DT.float32r
    bf16 = DT.bfloat16
    fp8 = DT.float8e4

    xt = nc.dram_tensor("xt", [KT, 128, T], bf16, kind="ExternalInput")
    wq = nc.dram_tensor("wq", [HPC, 128, KT, 128], bf16, kind="ExternalInput")
    wk = nc.dram_tensor("wk", [HPC, 128, KT, 128], bf16, kind="ExternalInput")
    wv = nc.dram_tensor("wv", [HPC // 2, 128, KT, 256], bf16, kind="ExternalInput")
    wo = nc.dram_tensor("wo", [128, HPC, D], bf16, kind="ExternalInput")
    bq = nc.dram_tensor("bq", [128, HPC], f32, kind="ExternalInput")
    bk = nc.dram_tensor("bk", [128, HPC], f32, kind="ExternalInput")
    maskadd = nc.dram_tensor("maskadd", [128, 1024], f32, kind="ExternalInput")
    ones8 = nc.dram_tensor("ones8", [128, 2, 128], fp8, kind="ExternalInput")
    ones64 = nc.dram_tensor("ones64", [128, 2, 2], fp8, kind="ExternalInput")
    onesr = nc.dram_tensor("onesr", [1, 128], f32r, kind="ExternalInput")
    outp = nc.dram_tensor("out", [T, D], f32, kind="ExternalOutput")

    def emit_rep(tc):
        import contextlib
        es = contextlib.ExitStack()
        with es:
            xt_pool = es.enter_context(tc.tile_pool(name="xt_pool", bufs=1))
            const_pool = es.enter_context(tc.tile_pool(name="const_pool", bufs=1))
            wqk_pool = es.enter_context(tc.tile_pool(name="wqk_pool", bufs=2))
            wv_pool = es.enter_context(tc.tile_pool(name="wv_pool", bufs=1))
            qk_pool = es.enter_context(tc.tile_pool(name="qk_pool", bufs=1))
            v8_pool = es.enter_context(tc.tile_pool(name="v8_pool", bufs=1))
            r8_pool = es.enter_context(tc.tile_pool(name="r8_pool", bufs=1))
            vbf_pool = es.enter_context(tc.tile_pool(name="vbf_pool", bufs=1))
            exm_pool = es.enter_context(tc.tile_pool(name="exm_pool", bufs=4))
            d8_pool = es.enter_context(tc.tile_pool(name="d8_pool", bufs=4))
            stg_pool = es.enter_context(tc.tile_pool(name="stg_pool", bufs=2))
            sm_pool = es.enter_context(tc.tile_pool(name="sm_pool", bufs=2))
            o_pool = es.enter_context(tc.tile_pool(name="o_pool", bufs=1))
            # ---- constants ------------------------------------------------
            bq_sb = const_pool.tile([128, HPC], f32)
            bk_sb = const_pool.tile([128, HPC], f32)
            mask_sb = const_pool.tile([128, 1024], f32)
            ones8_sb = const_pool.tile([128, 2, 128], fp8)
            ones64_sb = const_pool.tile([128, 2, 2], fp8)
            onesr_sb = const_pool.tile([1, 128], f32r)
            ln64_sb = const_pool.tile([128, 1], f32)
            nc.vector.memset(ln64_sb[:], LN64)
            onesb_sb = const_pool.tile([128, 128], bf16)
            nc.vector.memset(onesb_sb[:], 1.0)
            nc.sync.dma_start(out=bq_sb[:], in_=bq.ap())
            nc.sync.dma_start(out=bk_sb[:], in_=bk.ap())
            nc.sync.dma_start(out=mask_sb[:], in_=maskadd.ap())
            nc.sync.dma_start(out=ones8_sb[:], in_=ones8.ap())
            nc.sync.dma_start(out=ones64_sb[:], in_=ones64.ap())
            nc.sync.dma_start(out=onesr_sb[:], in_=onesr.ap())

            xts = []
            for a in range(KT):
                xta = xt_pool.tile([128, T], bf16, tag=f"xt{a}", name=f"xt{a}")
                nc.sync.dma_start(out=xta[:], in_=xt.ap()[a])
                xts.append(xta)

            o_sb = o_pool.tile([128, HPC, T], bf16)

            es2 = contextlib.ExitStack()
            with es2:
                ps_a = es2.enter_context(tc.tile_pool(name="ps_a", bufs=1, space="PSUM"))
                ps_s = es2.enter_context(tc.tile_pool(name="ps_s", bufs=3, space="PSUM"))
                ps_ot = es2.enter_context(tc.tile_pool(name="ps_ot", bufs=2, space="PSUM"))
                ps_db = es2.enter_context(tc.tile_pool(name="ps_db", bufs=1, space="PSUM"))
                ps_fc = es2.enter_context(tc.tile_pool(name="ps_fc", bufs=1, space="PSUM"))
                for pr in range(HPC // 2):
                    heads = (2 * pr, 2 * pr + 1)
                    qk_tiles = {}
                    # ---- A: q/k projections for both heads ----------------
                    for h in heads:
                        for wnm, w_dram, b_sb in (
                            ("q", wq, bq_sb), ("k", wk, bk_sb)
                        ):
                            wcol = wqk_pool.tile(
                                [128, KT, 128], bf16, tag="wcol"
                            )
                            nc.sync.dma_start(out=wcol[:], in_=w_dram.ap()[h])
                            dst = qk_pool.tile(
                                [128, T], bf16, tag=f"{wnm}T{h % 2}"
                            )
                            for c in range(TQ):
                                ps = ps_a.tile([128, 512], f32, tag="psa")
                                for a in range(KT):
                                    nc.tensor.matmul(
                                        ps[:],
                                        wcol[:, a, :],
                                        xts[a][:, c * 512:(c + 1) * 512],
                                        start=(a == 0),
                                        stop=(a == KT - 1),
                                    )
                                nc.scalar.activation(
                                    dst[:, c * 512:(c + 1) * 512], ps[:],
                                    AF.Identity, bias=b_sb[:, h:h + 1],
                                )
                            qk_tiles[(wnm, h)] = dst

                    # ---- A: v projection for the pair ---------------------
                    wvq = wv_pool.tile([128, KT, 256], bf16, tag="wvq")
                    nc.sync.dma_start(out=wvq[:], in_=wv.ap()[pr])
                    v8 = v8_pool.tile([128, TT, 256], fp8, tag="v8")
                    r8 = r8_pool.tile([128, TT, 256], fp8, tag="r8")
                    vbf = vbf_pool.tile([128, TT, 256], bf16, tag="vbf")
                    for tt in range(TT):
                        psf = ps_a.tile([128, 512], f32, tag="psa")
                        ps = psf[:, 0:256]
                        for a in range(KT):
                            nc.tensor.matmul(
                                ps,
                                xts[a][:, tt * 128:(tt + 1) * 128],
                                wvq[:, a, :],
                                start=(a == 0),
                                stop=(a == KT - 1),
                            )
                        with nc.allow_low_precision(
                            reason="fp8 v with explicit residual correction"
                        ):
                            nc.scalar.activation(
                                v8[:, tt, :], ps, AF.Identity, scale=32.0
                            )
                            nc.scalar.activation(
                                vbf[:, tt, :], ps, AF.Identity, scale=32.0
                            )
                            stg = stg_pool.tile([128, 256], bf16, tag="rstg")
                            # stg = v8/32 - v ;  r8 = -2048*stg = 2048*(v - v8/32)
                            nc.vector.scalar_tensor_tensor(
                                stg[:], v8[:, tt, :], 1.0 / 32.0, ps,
                                op0=ALU.mult, op1=ALU.subtract,
                            )
                            nc.vector.tensor_scalar_mul(
                                r8[:, tt, :], stg[:], -2048.0
                            )

                    # ---- B: attention per head ----------------------------
                    for h in heads:
                        qT = qk_tiles[("q", h)]
                        kT = qk_tiles[("k", h)]
                        hs = slice((h % 2) * 128, (h % 2) * 128 + 128)
                        for c in range(TQ):
                            ntk = 4 * (c + 1)
                            nd = 4 * c  # num strictly-below-diagonal tiles
                            qt = qT[:, c * 512:(c + 1) * 512]
                            otp = ps_ot.tile([128, 512], f32, tag="otp")
                            dbt = ps_db.tile([128, 512], f32, tag="dbt")
                            d8t = None
                            for j in range(ntk):
                                sp = ps_s.tile([128, 512], f32, tag="sp")
                                nc.tensor.matmul(
                                    sp[:],
                                    kT[:, j * 128:(j + 1) * 128],
                                    qt,
                                    start=True,
                                    stop=True,
                                )
                                d = j * 128 - c * 512
                                if d >= 0:
                                    nc.vector.tensor_add(
                                        sp[:], sp[:],
                                        mask_sb[:, 512 - d:1024 - d],
                                    )
                                exm = exm_pool.tile([128, 512], bf16, tag="exm")
                                nc.scalar.activation(
                                    exm[:], sp[:], AF.Exp,
                                    scale=SCALE, bias=ln64_sb[:],
                                )
                                if j < nd:
                                    if j % 2 == 0:
                                        d8t = d8_pool.tile(
                                            [128, 2, 512], fp8, tag="d8"
                                        )
                                    with nc.allow_low_precision(
                                        reason="fp8 softmax deltas by design"
                                    ):
                                        nc.vector.tensor_scalar_sub(
                                            d8t[:, j % 2, :], exm[:], 64.0
                                        )
                                if j >= nd:
                                    # diagonal tile: direct bf16 exm64 @ vbf32
                                    nc.tensor.matmul(
                                        otp[:],
                                        vbf[:, j, hs],
                                        exm[:],
                                        start=(j == 0),
                                        stop=(j == ntk - 1),
                                    )
                                if j % 2 == 1 and j < nd:
                                    # off-diag pair: fp8 DoubleRow
                                    nc.tensor.matmul(
                                        otp[:],
                                        v8[:, j - 1:j + 1, hs],
                                        d8t[:],
                                        start=(j == 1),
                                        stop=False,
                                        perf_mode=PM.DoubleRow,
                                    )
                                    nc.tensor.matmul(
                                        dbt[:, :],
                                        ones8_sb[:],
                                        d8t[:],
                                        start=(j == 1),
                                        stop=False,
                                        perf_mode=PM.DoubleRow,
                                    )
                                if j >= nd:
                                    # diag den: 64*sum(ex) via bf16 ones
                                    nc.tensor.matmul(
                                        dbt[:, :],
                                        onesb_sb[:],
                                        exm[:],
                                        start=(j == 0),
                                        stop=(j == ntk - 1),
                                    )
                            # FC: 32*sum(v) over kpos < 512c via v8 + r8
                            fc_sb = None
                            if nd > 0:
                                fcp = ps_fc.tile([128, 2], f32, tag="fcp")
                                for jj in range(0, nd, 2):
                                    nc.tensor.matmul(
                                        fcp[:],
                                        v8[:, jj:jj + 2, hs],
                                        ones8_sb[:, :, 0:2],
                                        start=(jj == 0),
                                        stop=False,
                                        perf_mode=PM.DoubleRow,
                                    )
                                for jj in range(0, nd, 2):
                                    nc.tensor.matmul(
                                        fcp[:],
                                        r8[:, jj:jj + 2, hs],
                                        ones64_sb[:],
                                        start=False,
                                        stop=(jj == nd - 2),
                                        perf_mode=PM.DoubleRow,
                                    )
                                fc_sb = sm_pool.tile([128, 1], f32, tag="fc")
                                nc.vector.tensor_copy(fc_sb[:], fcp[:, 0:1])
                            # den_sb = 32*den = 32*512*(c+1) + sum(d8)/2
                            den_sb = sm_pool.tile([1, 512], f32, tag="den")
                            nc.vector.tensor_scalar(
                                den_sb[:], dbt[0:1, :], 0.5,
                                float(32.0 * 512.0 * c),
                                op0=ALU.mult, op1=ALU.add,
                            )
                            rec = sm_pool.tile([1, 512], f32r, tag="rec")
                            with nc.allow_low_precision(
                                reason="f32r softmax reciprocal as in baseline"
                            ):
                                nc.vector.reciprocal(rec[:], den_sb[:])
                            # bc broadcast of rec into full dbt psum tile
                            nc.tensor.matmul(
                                dbt[:], onesr_sb[:], rec[:],
                                start=True, stop=True,
                            )
                            # t1 = otp/64 + fc  (= 32 * unnormalized ot)
                            t1 = stg_pool.tile([128, 512], f32, tag="t1")
                            if fc_sb is not None:
                                nc.vector.tensor_scalar(
                                    t1[:], otp[:], 1.0 / 64.0, fc_sb[:],
                                    op0=ALU.mult, op1=ALU.add,
                                )
                            else:
                                nc.vector.tensor_scalar_mul(
                                    t1[:], otp[:], 1.0 / 64.0
                                )
                            with nc.allow_low_precision(
                                reason="bf16 attention output by design"
                            ):
                                nc.vector.tensor_mul(
                                    o_sb[:, h, c * 512:(c + 1) * 512],
                                    t1[:], dbt[:],
                                )

            # ---- C: out = o @ wo ------------------------------------------
            es3 = contextlib.ExitStack()
            with es3:
                wo_pool = es3.enter_context(tc.tile_pool(name="wo_pool", bufs=2))
                co_stage = es3.enter_context(tc.tile_pool(name="co_stage", bufs=3))
                ps_c = es3.enter_context(tc.tile_pool(name="ps_c", bufs=2, space="PSUM"))
                for dc in range(4):
                    wot = wo_pool.tile([128, HPC, 512], bf16, tag="wot")
                    nc.sync.dma_start(
                        out=wot[:], in_=wo.ap()[:, :, dc * 512:(dc + 1) * 512]
                    )
                    for tt in range(TT):
                        psc = ps_c.tile([128, 512], f32, tag="psc")
                        for h in range(HPC):
                            nc.tensor.matmul(
                                psc[:],
                                o_sb[:, h, tt * 128:(tt + 1) * 128],
                                wot[:, h, :],
                                start=(h == 0),
                                stop=(h == HPC - 1),
                            )
                        stg = co_stage.tile([128, 512], f32, tag="cstg")
                        nc.scalar.activation(stg[:], psc[:], AF.Copy)
                        nc.sync.dma_start(
                            out=outp.ap()[tt * 128:(tt + 1) * 128,
                                          dc * 512:(dc + 1) * 512],
                            in_=stg[:],
                        )

    with TileContext(nc) as tc:
        for _rep in range(reps):
            emit_rep(tc)

    nc.compile()
    return nc


_NC_CACHE = {}


def _get_nc():
    if "nc" not in _NC_CACHE:
        _NC_CACHE["nc"] = build_nc()
    return _NC_CACHE["nc"]


def make_in_maps(query, w_q, b_q, w_k, b_k, w_v, b_v, w_o, b_o):
    query = np.asarray(query, dtype=np.float32)
    w_q = np.asarray(w_q, dtype=np.float32)
    w_k = np.asarray(w_k, dtype=np.float32)
    w_v = np.asarray(w_v, dtype=np.float32)
    w_o = np.asarray(w_o, dtype=np.float32)
    b_q = np.asarray(b_q, dtype=np.float32)
    b_k = np.asarray(b_k, dtype=np.float32)

    bf = ml_dtypes.bfloat16
    f8 = ml_dtypes.float8_e4m3

    g_idx = np.arange(1024)[None, :] - 512
    p_idx = np.arange(128)[:, None]
    maskadd = np.where(g_idx >= p_idx, 0.0, NEG).astype(np.float32)
    ones8 = np.ones((128, 2, 128), dtype=f8)
    ones64 = np.full((128, 2, 2), 1.0 / 64.0, dtype=f8)
    onesr = np.ones((1, 128), dtype=np.float32)

    in_maps = []
    for core in range(8):
        b = core // 2
        g = core % 2
        s = slice(g * 1024, (g + 1) * 1024)
        # xt[a, p, t] = query[b, t, 128a+p]
        xt = np.ascontiguousarray(
            query[b].T.reshape(KT, 128, T).astype(bf)
        )
        # wq[h, p, a, c] = w_q[128a+p, g*1024+128h+c]
        def prep_qk(w):
            wg = w[:, s].reshape(KT, 128, HPC, 128)  # [a, p, h, c]
            return np.ascontiguousarray(
                wg.transpose(2, 1, 0, 3).astype(bf)
            )
        # wv[pr, p, a, c] = w_v[128a+p, g*1024+256pr+c]
        wvg = w_v[:, s].reshape(KT, 128, HPC // 2, 256)
        wv_p = np.ascontiguousarray(wvg.transpose(2, 1, 0, 3).astype(bf))
        # wo[p, h, d] = w_o[g*1024+128h+p, d]
        wog = w_o[s, :].reshape(HPC, 128, D)
        wo_p = np.ascontiguousarray(wog.transpose(1, 0, 2).astype(bf))
        in_maps.append(
            {
                "xt": xt,
                "wq": prep_qk(w_q),
                "wk": prep_qk(w_k),
                "wv": wv_p,
                "wo": wo_p,
                "bq": np.ascontiguousarray(b_q[s].reshape(HPC, 128).T),
                "bk": np.ascontiguousarray(b_k[s].reshape(HPC, 128).T),
                "maskadd": maskadd,
                "ones8": ones8,
                "ones64": ones64,
                "onesr": onesr,
            }
        )

    return in_maps


def kernel(query, w_q, b_q, w_k, b_k, w_v, b_v, w_o, b_o, **kwargs):
    w_o = np.asarray(w_o, dtype=np.float32)
    b_v = np.asarray(b_v, dtype=np.float32)
    b_o = np.asarray(b_o, dtype=np.float32)
    in_maps = make_in_maps(query, w_q, b_q, w_k, b_k, w_v, b_v, w_o, b_o)
    global _LAST_IN_MAPS
    _LAST_IN_MAPS = in_maps
    nc = _get_nc()
    res = run_bass_kernel_spmd(nc, in_maps, core_ids=list(range(8)))

    out = np.zeros((B, T, D), dtype=np.float32)
    for core in range(8):
        out[core // 2] += res.results[core]["out"]
    out += (b_v @ w_o + b_o)[None, None, :]
    return out


# revision 11
# speedup vs baseline: 1.2232x; 1.1579x over previous
"""Causal self-attention Trainium2 kernel (fused, bf16 + fp8 delta attention).

Problem: B=4, T=2048, D=2048, H=16 heads x 128 head-size, fp32.
Sharding: 8 cores = 4 batches x 2 head-groups (8 heads each).

Per core, fully fused in SBUF (no DRAM spills):
  A: qT/kT = (x@w + b)^T in bf16, v in bf16(x32) + fp8(x32) + fp8 residual
  B: causal attention per head:
     S = kT^T q (bf16) -> exm64 = exp(S*scale + ln64) bf16
     delta8 = exm64 - 64 (fp8, exactly -64 at masked positions)
     OT psum (scale 2048*ex*v) = sum_offdiag DoubleRow(delta8 @ v8)
                               + sum_diag   exm64 @ vbf32
     FC psum (scale 32*sum v) = DoubleRow ones matmuls over v8 + r8 (residual)
     den psum = DoubleRow ones @ delta8 (exact -64 cancellation at masks)
     ot = (OT/64 + FC) * (1/(32*den))  -> o_sb bf16
  C: out = o_sb @ wo (bf16), f32 out.
Host sums the two group partials per batch and adds (b_v@w_o + b_o).
"""

import sys

sys.path.insert(0, "/opt/trn_rl_repo")

import numpy as np
import ml_dtypes

import concourse.bass as bass
import concourse.bacc as bacc
import concourse.mybir as mybir
from concourse.tile import TileContext
from concourse.bass_utils import run_bass_kernel_spmd

DT = mybir.dt
AF = mybir.ActivationFunctionType
ALU = mybir.AluOpType
PM = mybir.MatmulPerfMode

B, T, D = 4, 2048, 2048
HPC = 8                 # heads per core
DH = 128                # head size
KT = D // 128           # 16 contraction tiles
TQ = T // 512           # 4 query chunks of 512
TT = T // 128           # 16 t tiles
SCALE = 1.0 / np.sqrt(DH)
LN64 = float(np.log(64.0))
NEG = -1e10


def build_nc(reps=1):
    nc = bacc.Bacc("TRN2", target_bir_lowering=False, debug=False)
    f32 = DT.float32
    f32r = DT.float32r
    bf16 = DT.bfloat16
    fp8 = DT.float8e4

    xt = nc.dram_tensor("xt", [KT, 128, T], bf16, kind="ExternalInput")
    wq = nc.dram_tensor("wq", [HPC, 128, KT, 128], bf16, kind="ExternalInput")
    wk = nc.dram_tensor("wk", [HPC, 128, KT, 128], bf16, kind="ExternalInput")
    wv = nc.dram_tensor("wv", [HPC // 2, 128, KT, 256], bf16, kind="ExternalInput")
    wo = nc.dram_tensor("wo", [128, HPC, D], bf16, kind="ExternalInput")
    bq = nc.dram_tensor("bq", [128, HPC], f32, kind="ExternalInput")
    bk = nc.dram_tensor("bk", [128, HPC], f32, kind="ExternalInput")
    maskadd = nc.dram_tensor("maskadd", [128, 1024], f32, kind="ExternalInput")
    ones8 = nc.dram_tensor("ones8", [128, 2, 128], fp8, kind="ExternalInput")
    ones64 = nc.dram_tensor("ones64", [128, 2, 2], fp8, kind="ExternalInput")
    onesr = nc.dram_tensor("onesr", [1, 128], f32r, kind="ExternalInput")
    outp = nc.dram_tensor("out", [T, D], f32, kind="ExternalOutput")

    def emit_rep(tc):
        import contextlib
        es = contextlib.ExitStack()
        with es:
            xt_pool = es.enter_context(tc.tile_pool(name="xt_pool", bufs=1))
            const_pool = es.enter_context(tc.tile_pool(name="const_pool", bufs=1))
            wqk_pool = es.enter_context(tc.tile_pool(name="wqk_pool", bufs=2))
            wv_pool = es.enter_context(tc.tile_pool(name="wv_pool", bufs=1))
            qk_pool = es.enter_context(tc.tile_pool(name="qk_pool", bufs=1))
            v8_pool = es.enter_context(tc.tile_pool(name="v8_pool", bufs=1))
            r8_pool = es.enter_context(tc.tile_pool(name="r8_pool", bufs=1))
            vbf_pool = es.enter_context(tc.tile_pool(name="vbf_pool", bufs=1))
            exm_pool = es.enter_context(tc.tile_pool(name="exm_pool", bufs=6))
            d8_pool = es.enter_context(tc.tile_pool(name="d8_pool", bufs=6))
            stg_pool = es.enter_context(tc.tile_pool(name="stg_pool", bufs=3))
            sm_pool = es.enter_context(tc.tile_pool(name="sm_pool", bufs=2))
            o_pool = es.enter_context(tc.tile_pool(name="o_pool", bufs=1))
            # ---- constants ------------------------------------------------
            bq_sb = const_pool.tile([128, HPC], f32)
            bk_sb = const_pool.tile([128, HPC], f32)
            mask_sb = const_pool.tile([128, 1024], f32)
            ones8_sb = const_pool.tile([128, 2, 128], fp8)
            ones64_sb = const_pool.tile([128, 2, 2], fp8)
            onesr_sb = const_pool.tile([1, 128], f32r)
            ln64_sb = const_pool.tile([128, 1], f32)
            nc.vector.memset(ln64_sb[:], LN64)
            onesb_sb = const_pool.tile([128, 128], bf16)
            nc.vector.memset(onesb_sb[:], 1.0)
            nc.sync.dma_start(out=bq_sb[:], in_=bq.ap())
            nc.sync.dma_start(out=bk_sb[:], in_=bk.ap())
            nc.sync.dma_start(out=mask_sb[:], in_=maskadd.ap())
            nc.sync.dma_start(out=ones8_sb[:], in_=ones8.ap())
            nc.sync.dma_start(out=ones64_sb[:], in_=ones64.ap())
            nc.sync.dma_start(out=onesr_sb[:], in_=onesr.ap())

            xts = []
            for a in range(KT):
                xta = xt_pool.tile([128, T], bf16, tag=f"xt{a}", name=f"xt{a}")
                nc.sync.dma_start(out=xta[:], in_=xt.ap()[a])
                xts.append(xta)

            o_sb = o_pool.tile([128, HPC, T], bf16)

            es2 = contextlib.ExitStack()
            with es2:
                ps_a = es2.enter_context(tc.tile_pool(name="ps_a", bufs=2, space="PSUM"))
                ps_s = es2.enter_context(tc.tile_pool(name="ps_s", bufs=2, space="PSUM"))
                ps_ot = es2.enter_context(tc.tile_pool(name="ps_ot", bufs=2, space="PSUM"))
                ps_db = es2.enter_context(tc.tile_pool(name="ps_db", bufs=1, space="PSUM"))
                ps_fc = es2.enter_context(tc.tile_pool(name="ps_fc", bufs=1, space="PSUM"))
                for pr in range(HPC // 2):
                    heads = (2 * pr, 2 * pr + 1)
                    qk_tiles = {}
                    # ---- A: q/k projections for both heads ----------------
                    for h in heads:
                        for wnm, w_dram, b_sb in (
                            ("q", wq, bq_sb), ("k", wk, bk_sb)
                        ):
                            wcol = wqk_pool.tile(
                                [128, KT, 128], bf16, tag="wcol"
                            )
                            nc.sync.dma_start(out=wcol[:], in_=w_dram.ap()[h])
                            dst = qk_pool.tile(
                                [128, T], bf16, tag=f"{wnm}T{h % 2}"
                            )
                            for c in range(TQ):
                                ps = ps_a.tile([128, 512], f32, tag="psa")
                                for a in range(KT):
                                    nc.tensor.matmul(
                                        ps[:],
                                        wcol[:, a, :],
                                        xts[a][:, c * 512:(c + 1) * 512],
                                        start=(a == 0),
                                        stop=(a == KT - 1),
                                    )
                                nc.scalar.activation(
                                    dst[:, c * 512:(c + 1) * 512], ps[:],
                                    AF.Identity, bias=b_sb[:, h:h + 1],
                                )
                            qk_tiles[(wnm, h)] = dst

                    # ---- A: v projection for the pair ---------------------
                    wvq = wv_pool.tile([128, KT, 256], bf16, tag="wvq")
                    nc.sync.dma_start(out=wvq[:], in_=wv.ap()[pr])
                    v8 = v8_pool.tile([128, TT, 256], fp8, tag="v8")
                    r8 = r8_pool.tile([128, TT, 256], fp8, tag="r8")
                    vbf = vbf_pool.tile([128, TT, 256], bf16, tag="vbf")
                    for tt in range(TT):
                        psf = ps_a.tile([128, 512], f32, tag="psa")
                        ps = psf[:, 0:256]
                        for a in range(KT):
                            nc.tensor.matmul(
                                ps,
                                xts[a][:, tt * 128:(tt + 1) * 128],
                                wvq[:, a, :],
                                start=(a == 0),
                                stop=(a == KT - 1),
                            )
                        with nc.allow_low_precision(
                            reason="fp8 v with explicit residual correction"
                        ):
                            nc.scalar.activation(
                                v8[:, tt, :], ps, AF.Identity, scale=32.0
                            )
                            nc.scalar.activation(
                                vbf[:, tt, :], ps, AF.Identity, scale=32.0
                            )
                            stg = stg_pool.tile([128, 256], bf16, tag="rstg")
                            # stg = v8/32 - v ;  r8 = -2048*stg = 2048*(v - v8/32)
                            nc.vector.scalar_tensor_tensor(
                                stg[:], v8[:, tt, :], 1.0 / 32.0, ps,
                                op0=ALU.mult, op1=ALU.subtract,
                            )
                            nc.vector.tensor_scalar_mul(
                                r8[:, tt, :], stg[:], -2048.0
                            )

                    # ---- B: attention per head ----------------------------
                    for h in heads:
                        qT = qk_tiles[("q", h)]
                        kT = qk_tiles[("k", h)]
                        hs = slice((h % 2) * 128, (h % 2) * 128 + 128)
                        for c in range(TQ):
                            ntk = 4 * (c + 1)
                            nd = 4 * c  # num strictly-below-diagonal tiles
                            qt = qT[:, c * 512:(c + 1) * 512]
                            otp = ps_ot.tile([128, 512], f32, tag="otp")
                            dbt = ps_db.tile([128, 512], f32, tag="dbt")
                            d8t = None
                            for j in range(ntk):
                                sp = ps_s.tile([128, 512], f32, tag="sp")
                                nc.tensor.matmul(
                                    sp[:],
                                    kT[:, j * 128:(j + 1) * 128],
                                    qt,
                                    start=True,
                                    stop=True,
                                )
                                d = j * 128 - c * 512
                                if d >= 0:
                                    nc.vector.tensor_add(
                                        sp[:], sp[:],
                                        mask_sb[:, 512 - d:1024 - d],
                                    )
                                exm = exm_pool.tile([128, 512], bf16, tag="exm")
                                nc.scalar.activation(
                                    exm[:], sp[:], AF.Exp,
                                    scale=SCALE, bias=ln64_sb[:],
                                )
                                if j < nd:
                                    if j % 2 == 0:
                                        d8t = d8_pool.tile(
                                            [128, 2, 512], fp8, tag="d8"
                                        )
                                    with nc.allow_low_precision(
                                        reason="fp8 softmax deltas by design"
                                    ):
                                        nc.vector.tensor_scalar_sub(
                                            d8t[:, j % 2, :], exm[:], 64.0
                                        )
                                if j >= nd:
                                    # diagonal tile: direct bf16 exm64 @ vbf32
                                    nc.tensor.matmul(
                                        otp[:],
                                        vbf[:, j, hs],
                                        exm[:],
                                        start=(j == 0),
                                        stop=(j == ntk - 1),
                                    )
                                if j % 2 == 1 and j < nd:
                                    # off-diag pair: fp8 DoubleRow
                                    nc.tensor.matmul(
                                        otp[:],
                                        v8[:, j - 1:j + 1, hs],
                                        d8t[:],
                                        start=(j == 1),
                                        stop=False,
                                        perf_mode=PM.DoubleRow,
                                    )
                                    nc.tensor.matmul(
                                        dbt[:, :],
                                        ones8_sb[:],
                                        d8t[:],
                                        start=(j == 1),
                                        stop=False,
                                        perf_mode=PM.DoubleRow,
                                    )
                                if j >= nd:
                                    # diag den: 64*sum(ex) via bf16 ones
                                    nc.tensor.matmul(
                                        dbt[:, :],
                                        onesb_sb[:],
                                        exm[:],
                                        start=(j == 0),
                                        stop=(j == ntk - 1),
                                    )
                            # FC: 32*sum(v) over kpos < 512c via v8 + r8
                            fc_sb = None
                            if nd > 0:
                                fcp = ps_fc.tile([128, 2], f32, tag="fcp")
                                for jj in range(0, nd, 2):
                                    nc.tensor.matmul(
                                        fcp[:],
                                        v8[:, jj:jj + 2, hs],
                                        ones8_sb[:, :, 0:2],
                                        start=(jj == 0),
                                        stop=False,
                                        perf_mode=PM.DoubleRow,
                                    )
                                for jj in range(0, nd, 2):
                                    nc.tensor.matmul(
                                        fcp[:],
                                        r8[:, jj:jj + 2, hs],
                                        ones64_sb[:],
                                        start=False,
                                        stop=(jj == nd - 2),
                                        perf_mode=PM.DoubleRow,
                                    )
                                fc_sb = sm_pool.tile([128, 1], f32, tag="fc")
                                nc.vector.tensor_copy(fc_sb[:], fcp[:, 0:1])
                            # den_sb = 32*den = 32*512*(c+1) + sum(d8)/2
                            den_sb = sm_pool.tile([1, 512], f32, tag="den")
                            nc.vector.tensor_scalar(
                                den_sb[:], dbt[0:1, :], 0.5,
                                float(32.0 * 512.0 * c),
                                op0=ALU.mult, op1=ALU.add,
                            )
                            rec = sm_pool.tile([1, 512], f32r, tag="rec")
                            with nc.allow_low_precision(
                                reason="f32r softmax reciprocal as in baseline"
                            ):
                                nc.vector.reciprocal(rec[:], den_sb[:])
                            # bc broadcast of rec into full dbt psum tile
                            nc.tensor.matmul(
                                dbt[:], onesr_sb[:], rec[:],
                                start=True, stop=True,
                            )
                            # t1 = otp/64 + fc  (= 32 * unnormalized ot)
                            t1 = stg_pool.tile([128, 512], f32, tag="t1")
                            if fc_sb is not None:
                                nc.vector.tensor_scalar(
                                    t1[:], otp[:], 1.0 / 64.0, fc_sb[:],
                                    op0=ALU.mult, op1=ALU.add,
                                )
                            else:
                                nc.vector.tensor_scalar_mul(
                                    t1[:], otp[:], 1.0 / 64.0
                                )
                            with nc.allow_low_precision(
                                reason="bf16 attention output by design"
                            ):
                                nc.vector.tensor_mul(
                                    o_sb[:, h, c * 512:(c + 1) * 512],
                                    t1[:], dbt[:],
                                )

            # ---- C: out = o @ wo ------------------------------------------
            es3 = contextlib.ExitStack()
            with es3:
                wo_pool = es3.enter_context(tc.tile_pool(name="wo_pool", bufs=2))
                co_stage = es3.enter_context(tc.tile_pool(name="co_stage", bufs=3))
                ps_c = es3.enter_context(tc.tile_pool(name="ps_c", bufs=2, space="PSUM"))
                for dc in range(4):
                    wot = wo_pool.tile([128, HPC, 512], bf16, tag="wot")
                    nc.sync.dma_start(
                        out=wot[:], in_=wo.ap()[:, :, dc * 512:(dc + 1) * 512]
                    )
                    for tt in range(TT):
                        psc = ps_c.tile([128, 512], f32, tag="psc")
                        for h in range(HPC):
                            nc.tensor.matmul(
                                psc[:],
                                o_sb[:, h, tt * 128:(tt + 1) * 128],
                                wot[:, h, :],
                                start=(h == 0),
                                stop=(h == HPC - 1),
                            )
                        stg = co_stage.tile([128, 512], f32, tag="cstg")
                        nc.scalar.activation(stg[:], psc[:], AF.Copy)
                        nc.sync.dma_start(
                            out=outp.ap()[tt * 128:(tt + 1) * 128,
                                          dc * 512:(dc + 1) * 512],
                            in_=stg[:],
                        )

    with TileContext(nc) as tc:
        for _rep in range(reps):
            emit_rep(tc)

    nc.compile()
    return nc


_NC_CACHE = {}


def _get_nc():
    if "nc" not in _NC_CACHE:
        _NC_CACHE["nc"] = build_nc()
    return _NC_CACHE["nc"]


def make_in_maps(query, w_q, b_q, w_k, b_k, w_v, b_v, w_o, b_o):
    query = np.asarray(query, dtype=np.float32)
    w_q = np.asarray(w_q, dtype=np.float32)
    w_k = np.asarray(w_k, dtype=np.float32)
    w_v = np.asarray(w_v, dtype=np.float32)
    w_o = np.asarray(w_o, dtype=np.float32)
    b_q = np.asarray(b_q, dtype=np.float32)
    b_k = np.asarray(b_k, dtype=np.float32)

    bf = ml_dtypes.bfloat16
    f8 = ml_dtypes.float8_e4m3

    g_idx = np.arange(1024)[None, :] - 512
    p_idx = np.arange(128)[:, None]
    maskadd = np.where(g_idx >= p_idx, 0.0, NEG).astype(np.float32)
    ones8 = np.ones((128, 2, 128), dtype=f8)
    ones64 = np.full((128, 2, 2), 1.0 / 64.0, dtype=f8)
    onesr = np.ones((1, 128), dtype=np.float32)

    in_maps = []
    for core in range(8):
        b = core // 2
        g = core % 2
        s = slice(g * 1024, (g + 1) * 1024)
        # xt[a, p, t] = query[b, t, 128a+p]
        xt = np.ascontiguousarray(
            query[b].T.reshape(KT, 128, T).astype(bf)
        )
        # wq[h, p, a, c] = w_q[128a+p, g*1024+128h+c]
        def prep_qk(w):
            wg = w[:, s].reshape(KT, 128, HPC, 128)  # [a, p, h, c]
            return np.ascontiguousarray(
                wg.transpose(2, 1, 0, 3).astype(bf)
            )
        # wv[pr, p, a, c] = w_v[128a+p, g*1024+256pr+c]
        wvg = w_v[:, s].reshape(KT, 128, HPC // 2, 256)
        wv_p = np.ascontiguousarray(wvg.transpose(2, 1, 0, 3).astype(bf))
        # wo[p, h, d] = w_o[g*1024+128h+p, d]
        wog = w_o[s, :].reshape(HPC, 128, D)
        wo_p = np.ascontiguousarray(wog.transpose(1, 0, 2).astype(bf))
        in_maps.append(
            {
                "xt": xt,
                "wq": prep_qk(w_q),
                "wk": prep_qk(w_k),
                "wv": wv_p,
                "wo": wo_p,
                "bq": np.ascontiguousarray(b_q[s].reshape(HPC, 128).T),
                "bk": np.ascontiguousarray(b_k[s].reshape(HPC, 128).T),
                "maskadd": maskadd,
                "ones8": ones8,
                "ones64": ones64,
                "onesr": onesr,
            }
        )

    return in_maps


def kernel(query, w_q, b_q, w_k, b_k, w_v, b_v, w_o, b_o, **kwargs):
    w_o = np.asarray(w_o, dtype=np.float32)
    b_v = np.asarray(b_v, dtype=np.float32)
    b_o = np.asarray(b_o, dtype=np.float32)
    in_maps = make_in_maps(query, w_q, b_q, w_k, b_k, w_v, b_v, w_o, b_o)
    global _LAST_IN_MAPS
    _LAST_IN_MAPS = in_maps
    nc = _get_nc()
    res = run_bass_kernel_spmd(nc, in_maps, core_ids=list(range(8)))

    out = np.zeros((B, T, D), dtype=np.float32)
    for core in range(8):
        out[core // 2] += res.results[core]["out"]
    out += (b_v @ w_o + b_o)[None, None, :]
    return out


# revision 12
# speedup vs baseline: 5.5330x; 4.5232x over previous
"""Causal self-attention Trainium2 kernel (fused, bf16 + fp8 delta attention).

Problem: B=4, T=2048, D=2048, H=16 heads x 128 head-size, fp32.
Sharding: 8 cores = 4 batches x 2 head-groups (8 heads each).

Per core, fully fused in SBUF (no DRAM spills):
  A: qT/kT = (x@w + b)^T in bf16, v in bf16(x32) + fp8(x32) + fp8 residual
  B: causal attention per head:
     S = kT^T q (bf16) -> exm64 = exp(S*scale + ln64) bf16
     delta8 = exm64 - 64 (fp8, exactly -64 at masked positions)
     OT psum (scale 2048*ex*v) = sum_offdiag DoubleRow(delta8 @ v8)
                               + sum_diag   exm64 @ vbf32
     FC psum (scale 32*sum v) = DoubleRow ones matmuls over v8 + r8 (residual)
     den psum = DoubleRow ones @ delta8 (exact -64 cancellation at masks)
     ot = (OT/64 + FC) * (1/(32*den))  -> o_sb bf16
  C: out = o_sb @ wo (bf16), f32 out.
Host sums the two group partials per batch and adds (b_v@w_o + b_o).
"""

import sys

sys.path.insert(0, "/opt/trn_rl_repo")

import numpy as np
import ml_dtypes

import concourse.bass as bass
import concourse.bacc as bacc
import concourse.mybir as mybir
from concourse.tile import TileContext
from concourse.bass_utils import run_bass_kernel_spmd

DT = mybir.dt
AF = mybir.ActivationFunctionType
ALU = mybir.AluOpType
PM = mybir.MatmulPerfMode

B, T, D = 4, 2048, 2048
HPC = 8                 # heads per core
DH = 128                # head size
KT = D // 128           # 16 contraction tiles
TQ = T // 512           # 4 query chunks of 512
TT = T // 128           # 16 t tiles
SCALE = 1.0 / np.sqrt(DH)
LN64 = float(np.log(64.0))
NEG = -1e10


def build_nc(reps=1):
    nc = bacc.Bacc("TRN2", target_bir_lowering=False, debug=False)
    f32 = DT.float32
    f32r = DT.float32r
    bf16 = DT.bfloat16
    fp8 = DT.float8e4

    xt = nc.dram_tensor("xt", [KT, 128, T], bf16, kind="ExternalInput")
    wq = nc.dram_tensor("wq", [HPC, 128, KT, 128], bf16, kind="ExternalInput")
    wk = nc.dram_tensor("wk", [HPC, 128, KT, 128], bf16, kind="ExternalInput")
    wv = nc.dram_tensor("wv", [HPC // 2, 128, KT, 256], bf16, kind="ExternalInput")
    wo = nc.dram_tensor("wo", [128, HPC, D], bf16, kind="ExternalInput")
    bq = nc.dram_tensor("bq", [128, HPC], f32, kind="ExternalInput")
    bk = nc.dram_tensor("bk", [128, HPC], f32, kind="ExternalInput")
    maskadd = nc.dram_tensor("maskadd", [128, 1024], f32, kind="ExternalInput")
    ones8 = nc.dram_tensor("ones8", [128, 2, 128], fp8, kind="ExternalInput")
    ones64 = nc.dram_tensor("ones64", [128, 2, 2], fp8, kind="ExternalInput")
    onesr = nc.dram_tensor("onesr", [1, 128], f32r, kind="ExternalInput")
    outp = nc.dram_tensor("out", [T, D], f32, kind="ExternalOutput")

    def emit_rep(tc):
        import contextlib
        es = contextlib.ExitStack()
        with es:
            xt_pool = es.enter_context(tc.tile_pool(name="xt_pool", bufs=1))
            const_pool = es.enter_context(tc.tile_pool(name="const_pool", bufs=1))
            wqk_pool = es.enter_context(tc.tile_pool(name="wqk_pool", bufs=2))
            wv_pool = es.enter_context(tc.tile_pool(name="wv_pool", bufs=1))
            qk_pool = es.enter_context(tc.tile_pool(name="qk_pool", bufs=1))
            v8_pool = es.enter_context(tc.tile_pool(name="v8_pool", bufs=1))
            r8_pool = es.enter_context(tc.tile_pool(name="r8_pool", bufs=1))
            vbf_pool = es.enter_context(tc.tile_pool(name="vbf_pool", bufs=1))
            exm_pool = es.enter_context(tc.tile_pool(name="exm_pool", bufs=4))
            d8_pool = es.enter_context(tc.tile_pool(name="d8_pool", bufs=4))
            stg_pool = es.enter_context(tc.tile_pool(name="stg_pool", bufs=2))
            sm_pool = es.enter_context(tc.tile_pool(name="sm_pool", bufs=2))
            o_pool = es.enter_context(tc.tile_pool(name="o_pool", bufs=1))
            # ---- constants ------------------------------------------------
            bq_sb = const_pool.tile([128, HPC], f32)
            bk_sb = const_pool.tile([128, HPC], f32)
            mask_sb = const_pool.tile([128, 1024], f32)
            ones8_sb = const_pool.tile([128, 2, 128], fp8)
            ones64_sb = const_pool.tile([128, 2, 2], fp8)
            onesr_sb = const_pool.tile([1, 128], f32r)
            ln64_sb = const_pool.tile([128, 1], f32)
            nc.vector.memset(ln64_sb[:], LN64)
            onesb_sb = const_pool.tile([128, 128], bf16)
            nc.vector.memset(onesb_sb[:], 1.0)
            nc.sync.dma_start(out=bq_sb[:], in_=bq.ap())
            nc.sync.dma_start(out=bk_sb[:], in_=bk.ap())
            nc.sync.dma_start(out=mask_sb[:], in_=maskadd.ap())
            nc.sync.dma_start(out=ones8_sb[:], in_=ones8.ap())
            nc.sync.dma_start(out=ones64_sb[:], in_=ones64.ap())
            nc.sync.dma_start(out=onesr_sb[:], in_=onesr.ap())

            xts = []
            for a in range(KT):
                xta = xt_pool.tile([128, T], bf16, tag=f"xt{a}", name=f"xt{a}")
                nc.sync.dma_start(out=xta[:], in_=xt.ap()[a])
                xts.append(xta)

            o_sb = o_pool.tile([128, HPC, T], bf16)

            es2 = contextlib.ExitStack()
            with es2:
                ps_a = es2.enter_context(tc.tile_pool(name="ps_a", bufs=2, space="PSUM"))
                ps_s = es2.enter_context(tc.tile_pool(name="ps_s", bufs=2, space="PSUM"))
                ps_ot = es2.enter_context(tc.tile_pool(name="ps_ot", bufs=2, space="PSUM"))
                ps_db = es2.enter_context(tc.tile_pool(name="ps_db", bufs=1, space="PSUM"))
                ps_fc = es2.enter_context(tc.tile_pool(name="ps_fc", bufs=1, space="PSUM"))
                for pr in range(HPC // 2):
                    heads = (2 * pr, 2 * pr + 1)
                    qk_tiles = {}
                    # ---- A: q/k projections for both heads ----------------
                    for h in heads:
                        for wnm, w_dram, b_sb in (
                            ("q", wq, bq_sb), ("k", wk, bk_sb)
                        ):
                            wcol = wqk_pool.tile(
                                [128, KT, 128], bf16, tag="wcol"
                            )
                            nc.sync.dma_start(out=wcol[:], in_=w_dram.ap()[h])
                            dst = qk_pool.tile(
                                [128, T], bf16, tag=f"{wnm}T{h % 2}"
                            )
                            for c in range(TQ):
                                ps = ps_a.tile([128, 512], f32, tag="psa")
                                for a in range(KT):
                                    nc.tensor.matmul(
                                        ps[:],
                                        wcol[:, a, :],
                                        xts[a][:, c * 512:(c + 1) * 512],
                                        start=(a == 0),
                                        stop=(a == KT - 1),
                                    )
                                nc.scalar.activation(
                                    dst[:, c * 512:(c + 1) * 512], ps[:],
                                    AF.Identity, bias=b_sb[:, h:h + 1],
                                )
                            qk_tiles[(wnm, h)] = dst

                    # ---- A: v projection for the pair ---------------------
                    wvq = wv_pool.tile([128, KT, 256], bf16, tag="wvq")
                    nc.sync.dma_start(out=wvq[:], in_=wv.ap()[pr])
                    v8 = v8_pool.tile([128, TT, 256], fp8, tag="v8")
                    r8 = r8_pool.tile([128, TT, 256], fp8, tag="r8")
                    vbf = vbf_pool.tile([128, TT, 256], bf16, tag="vbf")
                    for tt in range(TT):
                        psf = ps_a.tile([128, 512], f32, tag="psa")
                        ps = psf[:, 0:256]
                        for a in range(KT):
                            nc.tensor.matmul(
                                ps,
                                xts[a][:, tt * 128:(tt + 1) * 128],
                                wvq[:, a, :],
                                start=(a == 0),
                                stop=(a == KT - 1),
                            )
                        with nc.allow_low_precision(
                            reason="fp8 v with explicit residual correction"
                        ):
                            nc.scalar.activation(
                                v8[:, tt, :], ps, AF.Identity, scale=32.0
                            )
                            nc.scalar.activation(
                                vbf[:, tt, :], ps, AF.Identity, scale=32.0
                            )
                            stg = stg_pool.tile([128, 256], bf16, tag="rstg")
                            # stg = v8/32 - v ;  r8 = -2048*stg = 2048*(v - v8/32)
                            nc.vector.scalar_tensor_tensor(
                                stg[:], v8[:, tt, :], 1.0 / 32.0, ps,
                                op0=ALU.mult, op1=ALU.subtract,
                            )
                            nc.vector.tensor_scalar_mul(
                                r8[:, tt, :], stg[:], -2048.0
                            )

                    # ---- B: attention per head ----------------------------
                    for h in heads:
                        qT = qk_tiles[("q", h)]
                        kT = qk_tiles[("k", h)]
                        hs = slice((h % 2) * 128, (h % 2) * 128 + 128)
                        for c in range(TQ):
                            ntk = 4 * (c + 1)
                            nd = 4 * c  # num strictly-below-diagonal tiles
                            qt = qT[:, c * 512:(c + 1) * 512]
                            otp = ps_ot.tile([128, 512], f32, tag="otp")
                            dbt = ps_db.tile([128, 512], f32, tag="dbt")
                            d8t = None
                            for j in range(ntk):
                                sp = ps_s.tile([128, 512], f32, tag="sp")
                                nc.tensor.matmul(
                                    sp[:],
                                    kT[:, j * 128:(j + 1) * 128],
                                    qt,
                                    start=True,
                                    stop=True,
                                )
                                d = j * 128 - c * 512
                                if d >= 0:
                                    nc.vector.tensor_add(
                                        sp[:], sp[:],
                                        mask_sb[:, 512 - d:1024 - d],
                                    )
                                exm = exm_pool.tile([128, 512], bf16, tag="exm")
                                nc.scalar.activation(
                                    exm[:], sp[:], AF.Exp,
                                    scale=SCALE, bias=ln64_sb[:],
                                )
                                if j < nd:
                                    if j % 2 == 0:
                                        d8t = d8_pool.tile(
                                            [128, 2, 512], fp8, tag="d8"
                                        )
                                    with nc.allow_low_precision(
                                        reason="fp8 softmax deltas by design"
                                    ):
                                        nc.vector.tensor_scalar_sub(
                                            d8t[:, j % 2, :], exm[:], 64.0
                                        )
                                if j >= nd:
                                    # diagonal tile: direct bf16 exm64 @ vbf32
                                    nc.tensor.matmul(
                                        otp[:],
                                        vbf[:, j, hs],
                                        exm[:],
                                        start=(j == 0),
                                        stop=(j == ntk - 1),
                                    )
                                if j % 2 == 1 and j < nd:
                                    # off-diag pair: fp8 DoubleRow
                                    nc.tensor.matmul(
                                        otp[:],
                                        v8[:, j - 1:j + 1, hs],
                                        d8t[:],
                                        start=(j == 1),
                                        stop=False,
                                        perf_mode=PM.DoubleRow,
                                    )
                                    nc.tensor.matmul(
                                        dbt[:, :],
                                        ones8_sb[:],
                                        d8t[:],
                                        start=(j == 1),
                                        stop=False,
                                        perf_mode=PM.DoubleRow,
                                    )
                                if j >= nd:
                                    # diag den: 64*sum(ex) via bf16 ones
                                    nc.tensor.matmul(
                                        dbt[:, :],
                                        onesb_sb[:],
                                        exm[:],
                                        start=(j == 0),
                                        stop=(j == ntk - 1),
                                    )
                            # FC: 32*sum(v) over kpos < 512c via v8 + r8
                            fc_sb = None
                            if nd > 0:
                                fcp = ps_fc.tile([128, 2], f32, tag="fcp")
                                for jj in range(0, nd, 2):
                                    nc.tensor.matmul(
                                        fcp[:],
                                        v8[:, jj:jj + 2, hs],
                                        ones8_sb[:, :, 0:2],
                                        start=(jj == 0),
                                        stop=False,
                                        perf_mode=PM.DoubleRow,
                                    )
                                for jj in range(0, nd, 2):
                                    nc.tensor.matmul(
                                        fcp[:],
                                        r8[:, jj:jj + 2, hs],
                                        ones64_sb[:],
                                        start=False,
                                        stop=(jj == nd - 2),
                                        perf_mode=PM.DoubleRow,
                                    )
                                fc_sb = sm_pool.tile([128, 1], f32, tag="fc")
                                nc.vector.tensor_copy(fc_sb[:], fcp[:, 0:1])
                            # den_sb = 32*den = 32*512*(c+1) + sum(d8)/2
                            den_sb = sm_pool.tile([1, 512], f32, tag="den")
                            nc.vector.tensor_scalar(
                                den_sb[:], dbt[0:1, :], 0.5,
                                float(32.0 * 512.0 * c),
                                op0=ALU.mult, op1=ALU.add,
                            )
                            rec = sm_pool.tile([1, 512], f32r, tag="rec")
                            with nc.allow_low_precision(
                                reason="f32r softmax reciprocal as in baseline"
                            ):
                                nc.vector.reciprocal(rec[:], den_sb[:])
                            # bc broadcast of rec into full dbt psum tile
                            nc.tensor.matmul(
                                dbt[:], onesr_sb[:], rec[:],
                                start=True, stop=True,
                            )
                            # t1 = otp/64 + fc  (= 32 * unnormalized ot)
                            t1 = stg_pool.tile([128, 512], f32, tag="t1")
                            if fc_sb is not None:
                                nc.vector.tensor_scalar(
                                    t1[:], otp[:], 1.0 / 64.0, fc_sb[:],
                                    op0=ALU.mult, op1=ALU.add,
                                )
                            else:
                                nc.vector.tensor_scalar_mul(
                                    t1[:], otp[:], 1.0 / 64.0
                                )
                            with nc.allow_low_precision(
                                reason="bf16 attention output by design"
                            ):
                                nc.vector.tensor_mul(
                                    o_sb[:, h, c * 512:(c + 1) * 512],
                                    t1[:], dbt[:],
                                )

            # ---- C: out = o @ wo ------------------------------------------
            es3 = contextlib.ExitStack()
            with es3:
                wo_pool = es3.enter_context(tc.tile_pool(name="wo_pool", bufs=2))
                co_stage = es3.enter_context(tc.tile_pool(name="co_stage", bufs=3))
                ps_c = es3.enter_context(tc.tile_pool(name="ps_c", bufs=2, space="PSUM"))
                for dc in range(4):
                    wot = wo_pool.tile([128, HPC, 512], bf16, tag="wot")
                    nc.sync.dma_start(
                        out=wot[:], in_=wo.ap()[:, :, dc * 512:(dc + 1) * 512]
                    )
                    for tt in range(TT):
                        psc = ps_c.tile([128, 512], f32, tag="psc")
                        for h in range(HPC):
                            nc.tensor.matmul(
                                psc[:],
                                o_sb[:, h, tt * 128:(tt + 1) * 128],
                                wot[:, h, :],
                                start=(h == 0),
                                stop=(h == HPC - 1),
                            )
                        stg = co_stage.tile([128, 512], f32, tag="cstg")
                        nc.scalar.activation(stg[:], psc[:], AF.Copy)
                        nc.sync.dma_start(
                            out=outp.ap()[tt * 128:(tt + 1) * 128,
                                          dc * 512:(dc + 1) * 512],
                            in_=stg[:],
                        )

    with TileContext(nc) as tc:
        for _rep in range(reps):
            emit_rep(tc)

    nc.compile()
    return nc


_NC_CACHE = {}


def _get_nc():
    if "nc" not in _NC_CACHE:
        _NC_CACHE["nc"] = build_nc()
    return _NC_CACHE["nc"]


def make_in_maps(query, w_q, b_q, w_k, b_k, w_v, b_v, w_o, b_o):
    query = np.asarray(query, dtype=np.float32)
    w_q = np.asarray(w_q, dtype=np.float32)
    w_k = np.asarray(w_k, dtype=np.float32)
    w_v = np.asarray(w_v, dtype=np.float32)
    w_o = np.asarray(w_o, dtype=np.float32)
    b_q = np.asarray(b_q, dtype=np.float32)
    b_k = np.asarray(b_k, dtype=np.float32)

    bf = ml_dtypes.bfloat16
    f8 = ml_dtypes.float8_e4m3

    g_idx = np.arange(1024)[None, :] - 512
    p_idx = np.arange(128)[:, None]
    maskadd = np.where(g_idx >= p_idx, 0.0, NEG).astype(np.float32)
    ones8 = np.ones((128, 2, 128), dtype=f8)
    ones64 = np.full((128, 2, 2), 1.0 / 64.0, dtype=f8)
    onesr = np.ones((1, 128), dtype=np.float32)

    in_maps = []
    for core in range(8):
        b = core // 2
        g = core % 2
        s = slice(g * 1024, (g + 1) * 1024)
        # xt[a, p, t] = query[b, t, 128a+p]
        xt = np.ascontiguousarray(
            query[b].T.reshape(KT, 128, T).astype(bf)
        )
        # wq[h, p, a, c] = w_q[128a+p, g*1024+128h+c]
        def prep_qk(w):
            wg = w[:, s].reshape(KT, 128, HPC, 128)  # [a, p, h, c]
            return np.ascontiguousarray(
                wg.transpose(2, 1, 0, 3).astype(bf)
            )
        # wv[pr, p, a, c] = w_v[128a+p, g*1024+256pr+c]
        wvg = w_v[:, s].reshape(KT, 128, HPC // 2, 256)
        wv_p = np.ascontiguousarray(wvg.transpose(2, 1, 0, 3).astype(bf))
        # wo[p, h, d] = w_o[g*1024+128h+p, d]
        wog = w_o[s, :].reshape(HPC, 128, D)
        wo_p = np.ascontiguousarray(wog.transpose(1, 0, 2).astype(bf))
        in_maps.append(
            {
                "xt": xt,
                "wq": prep_qk(w_q),
                "wk": prep_qk(w_k),
                "wv": wv_p,
                "wo": wo_p,
                "bq": np.ascontiguousarray(b_q[s].reshape(HPC, 128).T),
                "bk": np.ascontiguousarray(b_k[s].reshape(HPC, 128).T),
                "maskadd": maskadd,
                "ones8": ones8,
                "ones64": ones64,
                "onesr": onesr,
            }
        )

    return in_maps


def kernel(query, w_q, b_q, w_k, b_k, w_v, b_v, w_o, b_o, **kwargs):
    w_o = np.asarray(w_o, dtype=np.float32)
    b_v = np.asarray(b_v, dtype=np.float32)
    b_o = np.asarray(b_o, dtype=np.float32)
    in_maps = make_in_maps(query, w_q, b_q, w_k, b_k, w_v, b_v, w_o, b_o)
    global _LAST_IN_MAPS
    _LAST_IN_MAPS = in_maps
    nc = _get_nc()
    res = run_bass_kernel_spmd(nc, in_maps, core_ids=list(range(8)))

    out = np.zeros((B, T, D), dtype=np.float32)
    for core in range(8):
        out[core // 2] += res.results[core]["out"]
    out += (b_v @ w_o + b_o)[None, None, :]
    return out
